# revision 19
# baseline (speedup 1.0000x reference)
"""Multi-head differential attention on 8 Trainium2 NeuronCores.

Sharding: data-parallel over batch (B=2) x tensor-parallel over heads
(16 heads -> 4 per core). Core c handles batch c//4 and heads
4*(c%4) .. 4*(c%4)+3. Each core computes its heads' attention output and a
partial output projection; the host sums the 4 partials per batch.

v3 over v2: softmax denominators accumulated elementwise on the vector
engine (one ones-matmul per (head, block) instead of one per key chunk,
-100K PE cycles), DMA split across four per-engine hardware queues so the
weight/const stream never blocks the x stream (v2 pushed all 51MB through
the single Sync queue, stalling startup ~12us and each mode transition),
kc-outer projection loops so the first matmuls start as soon as the first
chunk of wq/xq lands, and the output projection spread one 512-col block
at a time between attention iterations as PE filler instead of 14us
bursts.
"""

import math
import os
import sys

sys.path.insert(0, "/opt/trn_rl_repo")

import numpy as np

B, S, HID, NH = 2, 2048, 2048, 16
HD = HID // NH          # 128
QKD = HD // 2           # 64
NCORES = 8
GRPS = NCORES // B      # head groups per batch
HPC = NH // GRPS        # heads per core = 4
LAYER_ID = 1
LAMBDA_INIT = 0.8 - 0.6 * math.exp(-0.3 * LAYER_ID)
EPS = 1e-6

NB = S // 512           # 4 seq blocks of 512
NKC = S // 128          # 16 key chunks of 128

_PROGRAM = None         # compiled bass program, reused across calls


def _build_program():
    import concourse.bass as bass
    import concourse.tile as tile
    from concourse import bacc, bass_isa, mybir

    f32 = mybir.dt.float32
    bf16 = mybir.dt.bfloat16
    Alu = mybir.AluOpType
    Act = mybir.ActivationFunctionType

    nc = bacc.Bacc(None, target_bir_lowering=False, debug=False)

    def din(name, shape, dt=bf16):
        return nc.dram_tensor(name, shape, dt, kind="ExternalInput").ap()

    io = {
        "xq_t": din("xq_t", [HID, S]),
        "xk_t": din("xk_t", [HID, S]),
        "xv_t": din("xv_t", [HID, S]),
        "wq_t": din("wq_t", [HID, 512]),
        "wk_t": din("wk_t", [HID, 512]),
        "wv_t": din("wv_t", [HID, 512]),
        "wo_t": din("wo_t", [512, HID]),
        "crep": din("crep", [128, S]),
        "srep": din("srep", [128, S]),
        "pmat": din("pmat", [128, 128]),
        "ones_a": din("ones_a", [128, 128]),
        "trimask": din("trimask", [128, 128]),
        "neglam": din("neglam", [128, 1], f32),
    }
    y_t = nc.dram_tensor("y_t", [HID, S], f32, kind="ExternalOutput").ap()

    from contextlib import ExitStack

    with tile.TileContext(nc) as tc, ExitStack() as ctx:
        persist = ctx.enter_context(tc.tile_pool(name="persist", bufs=1))
        constp = ctx.enter_context(tc.tile_pool(name="constp", bufs=1))

        # constants
        crep = constp.tile([128, S], bf16, name="crep_sb", tag="crep")
        srep = constp.tile([128, S], bf16, name="srep_sb", tag="srep")
        pmat = constp.tile([128, 128], bf16, name="pmat_sb", tag="pmat")
        ones_a = constp.tile([128, 128], bf16, name="ones_a_sb", tag="ones_a")
        trimask = constp.tile([128, 128], bf16, name="trimask_sb", tag="trimask")
        neglam = constp.tile([128, 1], f32, name="neglam_sb", tag="neglam")

        # persistent tensors: Q^T/K^T per (map g, head-pair hp): [128, S]
        #   tile t = 2*g + hp; partitions [64*a, 64*a+64) hold head 2*hp+a.
        QT = [persist.tile([128, S], bf16, name=f"qt{t}", tag=f"qt{t}")
              for t in range(4)]
        KT = [persist.tile([128, S], bf16, name=f"kt{t}", tag=f"kt{t}")
              for t in range(4)]
        # V natural layout per 128-seq chunk: [128 seq, 4 heads * 128 feat]
        VH = [persist.tile([128, 512], bf16, name=f"vh{s}", tag=f"vh{s}")
              for s in range(NKC)]
        # combined attention output (post RMS): [feat, seq] per head
        U = [persist.tile([128, S], bf16, name=f"u{h}", tag=f"u{h}")
             for h in range(HPC)]
        wo = [persist.tile([128, S], bf16, name=f"wo{h}", tag=f"wo{h}")
              for h in range(HPC)]

        # ---------------- phase W: warmup --------------------------------
        # the PE clock drops to a low p-state when idle and takes ~3us of
        # continuous execution to ramp back up; run junk matmuls on memset
        # tiles during the initial DMA wait so the first real matmuls run at
        # full speed (and the wait itself is hidden)
        with tc.tile_pool(name="warm", bufs=1) as warm, \
             tc.tile_pool(name="wps", bufs=1, space="PSUM") as wps:
            jnk = warm.tile([128, 512], bf16, name="jnk", tag="jnk")
            nc.vector.memset(jnk[:], 0.0)
            jp = wps.tile([128, 512], f32, name="jp", tag="jp")
            for i in range(40):
                nc.tensor.matmul(jp[:], jnk[:, 0:128], jnk[:],
                                 start=(i == 0), stop=(i == 39))

        # ---------------- phase P: q/k/v projections + fused rope ----------
        with tc.tile_pool(name="wp", bufs=1) as wp, \
             tc.tile_pool(name="xp", bufs=2) as xp, \
             tc.tile_pool(name="rsb", bufs=2) as rsb, \
             tc.tile_pool(name="rawp", bufs=6) as rawp, \
             tc.tile_pool(name="pp", bufs=1, space="PSUM") as pp, \
             tc.tile_pool(name="pxp", bufs=2, space="PSUM") as pxp:
            wt = {}
            wnames = ("wq_t", "wk_t", "wv_t")

            # DMA queue split (only SP/Activation/gpsimd can issue DMAs):
            # weight chunks alternate between the scalar and gpsimd queues
            # (double stream rate for w0, which races the first blocks'
            # consumption), consts + wo on the scalar queue after w0, x
            # activations and (later, temporally disjoint) y output on the
            # sync queue.
            def load_weights(mi, eng_even, eng_odd):
                for kc in range(NKC):
                    w_ = wp.tile([128, 512], bf16, name=f"w{mi}_{kc}",
                                 tag=f"w{mi}_{kc}")
                    eng = eng_even if kc % 2 == 0 else eng_odd
                    eng.dma_start(
                        out=w_[:], in_=io[wnames[mi]][kc * 128:(kc + 1) * 128, :])
                    wt[(mi, kc)] = w_

            load_weights(0, nc.scalar, nc.gpsimd)
            for t, key in ((pmat, "pmat"), (trimask, "trimask"),
                           (neglam, "neglam"), (ones_a, "ones_a"),
                           (crep, "crep"), (srep, "srep")):
                nc.scalar.dma_start(out=t[:], in_=io[key][:])
            for h in range(HPC):
                nc.scalar.dma_start(out=wo[h][:],
                                    in_=io["wo_t"][h * 128:(h + 1) * 128, :])
            load_weights(1, nc.gpsimd, nc.scalar)
            load_weights(2, nc.gpsimd, nc.scalar)

            def load_x(mi, n):
                xin = io[("xq_t", "xk_t", "xv_t")[mi]]
                xt = []
                for kc in range(NKC):
                    xck = xp.tile([128, 512], bf16, name=f"x_{mi}_{n}_{kc}",
                                  tag=f"x{kc}")
                    nc.sync.dma_start(
                        out=xck[:],
                        in_=xin[kc * 128:(kc + 1) * 128,
                                n * 512:(n + 1) * 512])
                    xt.append(xck)
                return xt

            pending = []

            def flush_rope():
                T, t, n, raw = pending.pop(0)
                px = pxp.tile([128, 512], f32, name=f"px_{T[t].name}_{n}",
                              tag="px")
                nc.tensor.matmul(px[:], pmat[:], raw[:], start=True, stop=True)
                pxb = rsb.tile([128, 512], bf16, name=f"pxb_{T[t].name}_{n}",
                               tag="pxb")
                nc.scalar.copy(pxb[:], px[:])
                cs = slice(n * 512, (n + 1) * 512)
                tmp = rsb.tile([128, 512], bf16, name=f"tmp_{T[t].name}_{n}",
                               tag="tmp")
                nc.vector.tensor_mul(tmp[:], pxb[:], srep[:, cs])
                aa = rsb.tile([128, 512], bf16, name=f"aa_{T[t].name}_{n}",
                              tag="aa")
                nc.vector.tensor_mul(aa[:], raw[:], crep[:, cs])
                nc.vector.tensor_add(T[t][:, cs], aa[:], tmp[:])

            blocks = [(mi, n) for mi in range(3) for n in range(NB)]
            xts = {0: load_x(*blocks[0]), 1: load_x(*blocks[1])}
            for idx, (mi, n) in enumerate(blocks):
                xt = xts.pop(idx)
                if idx + 2 < len(blocks):
                    xts[idx + 2] = load_x(*blocks[idx + 2])
                ps = [pp.tile([128, 512], f32, name=f"pp{t}_{mi}_{n}",
                              tag=f"pp{t}") for t in range(4)]
                # kc-outer: each x/w chunk is consumed in 4 back-to-back
                # matmuls right after it lands, so the PE never waits for a
                # whole 16-chunk block of DMA before starting
                for kc in range(NKC):
                    for t in range(4):
                        nc.tensor.matmul(
                            ps[t][:],
                            wt[(mi, kc)][:, t * 128:(t + 1) * 128]
                            if mi < 2 else xt[kc][:, t * 128:(t + 1) * 128],
                            xt[kc][:] if mi < 2 else wt[(mi, kc)][:],
                            start=(kc == 0), stop=(kc == 15))
                    if kc in (2, 6, 10, 14) and pending:
                        flush_rope()
                for t in range(4):
                    if mi == 2:
                        nc.scalar.copy(VH[n * 4 + t][:], ps[t][:])
                    else:
                        raw = rawp.tile([128, 512], bf16,
                                        name=f"raw_{mi}_{n}_{t}", tag="raw")
                        # on scalar: in the vector queue this cast sits
                        # behind rope multiplies, and the next block's
                        # psum-tag WAR stalls the PE on it
                        nc.scalar.copy(raw[:], ps[t][:])
                        pending.append((QT if mi == 0 else KT, t, n, raw))
            while pending:
                flush_rope()

        # ---------------- phase A: attention + rms + output proj ----------
        with tc.tile_pool(name="sp", bufs=2, space="PSUM") as sp, \
             tc.tile_pool(name="pvp", bufs=1, space="PSUM") as pvp, \
             tc.tile_pool(name="op", bufs=2, space="PSUM") as op, \
             tc.tile_pool(name="rsp", bufs=2) as rsp, \
             tc.tile_pool(name="ep", bufs=5) as ep, \
             tc.tile_pool(name="cb", bufs=2) as cb, \
             tc.tile_pool(name="ys", bufs=2) as ys:
            pending_rms = []
            pending_sm = []
            i32 = mybir.dt.int32

            def flush_rms():
                # rms tail for a completed (h, qb): partition-sum of U^2 on
                # the PE, then rstd = rsqrt(mean+eps) via the exponent
                # bit-trick + one Newton step on the vector engine.
                h_, qb_, sq_, dst_ = pending_rms.pop()
                ssq_t = op.tile([128, 512], f32, name=f"ssq_{h_}_{qb_}",
                                tag="o")
                nc.tensor.matmul(ssq_t[:], ones_a[:], sq_[:],
                                 start=True, stop=True)
                m_ = cb.tile([128, 512], f32, name=f"m_{h_}_{qb_}", tag="m")
                # on vector, not scalar: the scalar queue is exp-saturated in
                # phase A and any extra op there delays the exp chain, which
                # WAR-stalls the PE on the scores double-buffer
                nc.vector.tensor_scalar(m_[:], ssq_t[:], 1.0 / HD, EPS,
                                        op0=Alu.mult, op1=Alu.add)
                y0 = cb.tile([128, 512], f32, name=f"y0_{h_}_{qb_}", tag="y0")
                nc.vector.tensor_scalar(
                    y0[:].bitcast(i32), m_[:].bitcast(i32), 1, -1,
                    op0=Alu.logical_shift_right, op1=Alu.bitwise_xor)
                nc.vector.tensor_scalar(
                    y0[:].bitcast(i32), y0[:].bitcast(i32), 0x5f3759e0, None,
                    op0=Alu.add)
                uu = cb.tile([128, 512], f32, name=f"uu_{h_}_{qb_}", tag="uu")
                nc.vector.tensor_mul(uu[:], y0[:], y0[:])
                nc.vector.scalar_tensor_tensor(
                    uu[:], m_[:], -0.5, uu[:], op0=Alu.mult, op1=Alu.mult)
                nc.vector.tensor_scalar(uu[:], uu[:], 1.5, None, op0=Alu.add)
                rstdb = cb.tile([128, 512], bf16, name=f"rstdb_{h_}_{qb_}",
                                tag="rstdb")
                nc.vector.tensor_mul(rstdb[:], y0[:], uu[:])
                nc.vector.tensor_mul(dst_, dst_, rstdb[:])

            def oproj_unit(qb, pr, i):
                # one 128-row x 512-query slice of the output projection for
                # completed block qb; spread between attention iterations as
                # PE filler
                oc = 2 * pr + i
                psy = op.tile([128, 512], f32, name=f"py_{qb}_{oc}", tag="o")
                for h2 in range(HPC):
                    nc.tensor.matmul(
                        psy[:], wo[h2][:, oc * 128:(oc + 1) * 128],
                        U[h2][:, qb * 512:(qb + 1) * 512],
                        start=(h2 == 0), stop=(h2 == HPC - 1))
                yst = ys.tile([128, 512], f32, name=f"yst_{qb}_{oc}",
                              tag="yst")
                # psum->sbuf drain on vector, keeping scalar free for exp
                nc.vector.tensor_scalar(yst[:], psy[:], 0.0, None,
                                        op0=Alu.add)
                nc.sync.dma_start(
                    out=y_t[oc * 128:(oc + 1) * 128,
                            qb * 512:(qb + 1) * 512],
                    in_=yst[:])

            def flush_sm():
                # deferred softmax-denominator + combine chain for the
                # previous head, emitted two iterations into the next head's
                # loop so the PE never sits on the DVE drain at an h boundary
                h_, qb_, pv_, run_ = pending_sm.pop()
                rb = cb.tile([128, 1024], f32, name=f"rb_{h_}_{qb_}",
                             tag="rb")
                tt = cb.tile([128, 1024], f32, name=f"tt_{h_}_{qb_}",
                             tag="tt")
                for g in (0, 1):
                    sl = slice(g * 512, (g + 1) * 512)
                    # bf16 accumulator: each element only sums <=16 chunk
                    # values (the 128-key reduction happens in f32 psum), so
                    # rounding stays ~0.3% on the denominators
                    ps2 = op.tile([128, 512], f32, name=f"sm{g}_{h_}_{qb_}",
                                  tag="o")
                    nc.tensor.matmul(ps2[:], ones_a[:], run_[:, sl],
                                     start=True, stop=True)
                    nc.vector.reciprocal_approx_fast(rb[:, sl], ps2[:])
                    nc.vector.tensor_mul(tt[:, sl], pv_[:, sl], rb[:, sl])
                dst = U[h_][:, qb_ * 512:(qb_ + 1) * 512]
                nc.vector.scalar_tensor_tensor(
                    dst, tt[:, 512:1024], neglam[:], tt[:, 0:512],
                    op0=Alu.mult, op1=Alu.add)
                sq = cb.tile([128, 512], bf16, name=f"sq_{h_}_{qb_}",
                             tag="sq")
                nc.vector.tensor_mul(sq[:], dst, dst)
                pending_rms.append((h_, qb_, sq, dst))

            oq = []           # pending oproj units of the previous qb
            for qb in range(NB):
                nkc = 4 * qb + 4
                iters = HPC * nkc
                due = []
                if oq:
                    due = [7 + round(u * (iters - 9) / len(oq))
                           for u in range(len(oq))]
                it = 0
                for h in range(HPC):
                    hp, a = h // 2, h % 2
                    poff = 64 * a
                    pv = pvp.tile([128, 1024], f32, name=f"pv_{h}_{qb}",
                                  tag="pv")
                    run = rsp.tile([128, 1024], bf16, name=f"run_{h}_{qb}",
                                   tag="run")
                    pvq = []

                    def emit_pv(item):
                        E_, qoff_, first_, last_, kc_ = item
                        for g in (0, 1):
                            sl = slice(g * 512 + qoff_, (g + 1) * 512)
                            nc.tensor.matmul(
                                pv[:, sl],
                                VH[kc_][:, h * 128:(h + 1) * 128],
                                E_[:, sl], start=first_, stop=last_)

                    for kc in range(nkc):
                        j = kc - 4 * qb
                        qoff = max(j, 0) * 128
                        if kc == 1 and pending_sm:
                            flush_sm()
                        if kc == 3 and pending_rms:
                            flush_rms()
                        while oq and due and it >= due[0]:
                            due.pop(0)
                            oproj_unit(*oq.pop(0))
                        ps = sp.tile([128, 1024], f32, name=f"s_{h}_{qb}_{kc}",
                                     tag="s")
                        for g in (0, 1):
                            tq = 2 * g + hp
                            nc.tensor.matmul(
                                ps[:, g * 512 + qoff:(g + 1) * 512],
                                KT[tq][poff:poff + 64,
                                       kc * 128:(kc + 1) * 128],
                                QT[tq][poff:poff + 64,
                                       qb * 512 + qoff:(qb + 1) * 512],
                                start=True, stop=True)
                        E = ep.tile([128, 1024], bf16, name=f"e_{h}_{qb}_{kc}",
                                    tag="e")
                        if qoff == 0:
                            nc.scalar.activation(E[:], ps[:], Act.Exp,
                                                 scale=0.125)
                        else:
                            for g in (0, 1):
                                nc.scalar.activation(
                                    E[:, g * 512 + qoff:(g + 1) * 512],
                                    ps[:, g * 512 + qoff:(g + 1) * 512],
                                    Act.Exp, scale=0.125)
                        if j >= 0:
                            for g in (0, 1):
                                sl = E[:, g * 512 + qoff:g * 512 + qoff + 128]
                                # causal mask on the idle GpSimd engine
                                nc.gpsimd.tensor_mul(sl, sl, trimask[:])
                        # softmax denominator: elementwise chunk accumulation
                        # on the vector engine; partition-sum once per (h,qb)
                        if kc == 0:
                            nc.vector.tensor_scalar(run[:], E[:], 0.0, None,
                                                    op0=Alu.add)
                        elif qoff == 0:
                            nc.vector.tensor_add(run[:], run[:], E[:])
                        else:
                            for g in (0, 1):
                                sl = slice(g * 512 + qoff, (g + 1) * 512)
                                nc.vector.tensor_add(run[:, sl], run[:, sl],
                                                     E[:, sl])
                        pvq.append((E, qoff, kc == 0, kc == nkc - 1, kc))
                        if len(pvq) > 3:
                            emit_pv(pvq.pop(0))
                        it += 1
                    while pvq:
                        emit_pv(pvq.pop(0))
                    # combine U = pv1/sm1 - lam*pv2/sm2 is deferred into the
                    # next head's loop (flush_sm)
                    pending_sm.append((h, qb, pv, run))
                while oq:
                    oproj_unit(*oq.pop(0))
                if qb < NB - 1:
                    oq = [(qb, pr, i) for pr in range(8) for i in (0, 1)]
            # final block's output projection: h0-h2 contributions of two
            # chains first (hiding the deferred h3 sm+rms chains behind PE
            # work), h3 completions after
            qb = NB - 1

            def pyf_partial(grp):
                tiles = []
                for pr in (2 * grp, 2 * grp + 1):
                    psy = sp.tile([128, 1024], f32, name=f"pyf_{pr}", tag="s")
                    for i in (0, 1):
                        oc = 2 * pr + i
                        for h2 in range(HPC - 1):
                            nc.tensor.matmul(
                                psy[:, i * 512:(i + 1) * 512],
                                wo[h2][:, oc * 128:(oc + 1) * 128],
                                U[h2][:, qb * 512:(qb + 1) * 512],
                                start=(h2 == 0), stop=False)
                    tiles.append((pr, psy))
                return tiles

            for grp in range(4):
                tiles = pyf_partial(grp)
                if grp == 0:
                    # the h3 sm+rms chains drain on DVE while the first
                    # grp's h0-h2 matmuls keep the PE busy
                    while pending_sm:
                        flush_sm()
                    while pending_rms:
                        flush_rms()
                for pr, psy in tiles:
                    for i in (0, 1):
                        oc = 2 * pr + i
                        nc.tensor.matmul(
                            psy[:, i * 512:(i + 1) * 512],
                            wo[HPC - 1][:, oc * 128:(oc + 1) * 128],
                            U[HPC - 1][:, qb * 512:(qb + 1) * 512],
                            start=False, stop=True)
                    yst = ys.tile([128, 1024], f32, name=f"ystf_{pr}",
                                  tag="ysf")
                    nc.scalar.copy(yst[:], psy[:])
                    for i in (0, 1):
                        oc = 2 * pr + i
                        eng = nc.sync if pr % 2 == 0 else nc.scalar
                        eng.dma_start(
                            out=y_t[oc * 128:(oc + 1) * 128,
                                    qb * 512:(qb + 1) * 512],
                            in_=yst[:, i * 512:(i + 1) * 512])

    nc.compile()
    return nc


def _host_prep(q, k, v, Wq, Wk, Wv, Wo, lambda_q1, lambda_k1, lambda_q2,
               lambda_k2, gnorm_w, cos_emb, sin_emb):
    import ml_dtypes

    f32 = np.float32
    bf16 = ml_dtypes.bfloat16
    q = np.asarray(q, f32); k = np.asarray(k, f32); v = np.asarray(v, f32)
    Wq = np.asarray(Wq, f32); Wk = np.asarray(Wk, f32)
    Wv = np.asarray(Wv, f32); Wo = np.asarray(Wo, f32)
    gnorm_w = np.asarray(gnorm_w, f32)
    cos_emb = np.asarray(cos_emb, f32); sin_emb = np.asarray(sin_emb, f32)

    lam1 = np.exp(np.sum(np.asarray(lambda_q1, f32) * np.asarray(lambda_k1, f32),
                         dtype=f32))
    lam2 = np.exp(np.sum(np.asarray(lambda_q2, f32) * np.asarray(lambda_k2, f32),
                         dtype=f32))
    lam = np.float32(lam1 - lam2 + LAMBDA_INIT)

    # per-batch transposed activations (bf16)
    xt = {}
    for b in range(B):
        xt[("q", b)] = np.ascontiguousarray(q[b].T).astype(bf16)
        xt[("k", b)] = np.ascontiguousarray(k[b].T).astype(bf16)
        xt[("v", b)] = np.ascontiguousarray(v[b].T).astype(bf16)

    # shared constant tensors
    base_c = cos_emb[:S, :QKD]          # [S, 64]
    base_s = sin_emb[:S, :QKD]
    crep = np.ascontiguousarray(np.tile(base_c.T, (2, 1))).astype(bf16)
    srep = np.ascontiguousarray(np.tile(base_s.T, (2, 1))).astype(bf16)
    pmat = np.zeros((128, 128), f32)
    for blk in range(2):
        o = blk * 64
        for i in range(QKD // 2):
            pmat[o + 2 * i, o + 2 * i + 1] = 1.0     # lhsT[2i, 2i+1]
            pmat[o + 2 * i + 1, o + 2 * i] = -1.0    # lhsT[2i+1, 2i]
    pmat = pmat.astype(bf16)
    ones_a = np.ones((128, 128), f32).astype(bf16)
    # trimask[p, n] = 1 if key-in-chunk p <= query-in-block n (valid)
    trimask = np.triu(np.ones((128, 128), f32), 0).astype(bf16)
    neglam = np.full((128, 1), -lam, f32)

    per_core = []
    for c in range(NCORES):
        b, grp = c // GRPS, c % GRPS
        heads = [HPC * grp + j for j in range(HPC)]
        # wq/wk columns: tile t = 2*g + hp; within tile: head 2*hp+a at
        # cols [64*a, 64*a+64), original feature order (interleaved pairs)
        cols = []
        for t in range(4):
            g, hp = t // 2, t % 2
            for a2 in range(2):
                hg = heads[2 * hp + a2]
                cols.extend(hg * HD + g * QKD + d for d in range(QKD))
        cols = np.asarray(cols)
        vrows = np.asarray([h * HD + d for h in heads for d in range(HD)])
        wq_t = np.ascontiguousarray(Wq[cols, :].T).astype(bf16)
        wk_t = np.ascontiguousarray(Wk[cols, :].T).astype(bf16)
        wv_t = np.ascontiguousarray(Wv[vrows, :].T).astype(bf16)
        gtile = np.tile(gnorm_w, HPC)                       # [512]
        wo_t = np.ascontiguousarray(
            ((1.0 - LAMBDA_INIT) * Wo[:, vrows] * gtile[None, :]).T).astype(bf16)
        per_core.append({
            "xq_t": xt[("q", b)], "xk_t": xt[("k", b)], "xv_t": xt[("v", b)],
            "wq_t": wq_t, "wk_t": wk_t, "wv_t": wv_t, "wo_t": wo_t,
            "crep": crep, "srep": srep, "pmat": pmat,
            "ones_a": ones_a, "trimask": trimask, "neglam": neglam,
        })
    return per_core


def _install_ntff_hook():
    """antenv.axon_hooks is absent in this image; synthesize it so
    run_bass_kernel_spmd(trace=True) can capture NTFF profiles."""
    import sys as _sys
    import types

    if "antenv.axon_hooks" in _sys.modules:
        return
    import antenv
    mod = types.ModuleType("antenv.axon_hooks")
    state = {"hook": None}
    mod.set_axon_ntff_profile_hook = lambda h: state.__setitem__("hook", h)
    mod.get_axon_ntff_profile_hook = lambda: state["hook"]
    _sys.modules["antenv.axon_hooks"] = mod
    antenv.axon_hooks = mod
    try:
        from trn_agent_boot.trn_boot import _ntff_profile_via_ctypes
        state["hook"] = _ntff_profile_via_ctypes("/opt/axon/libaxon_pjrt.so")
    except Exception as e:  # degrade: trace skipped, run still works
        print("ntff hook install failed:", e)


def kernel(q, k, v, Wq, Wk, Wv, Wo, lambda_q1, lambda_k1, lambda_q2,
           lambda_k2, gnorm_w, cos_emb, sin_emb, mask, _trace=False):
    if _trace:
        _install_ntff_hook()
    global _PROGRAM
    if _PROGRAM is None:
        _PROGRAM = _build_program()
    nc = _PROGRAM

    in_maps = _host_prep(q, k, v, Wq, Wk, Wv, Wo, lambda_q1, lambda_k1,
                         lambda_q2, lambda_k2, gnorm_w, cos_emb, sin_emb)

    from concourse.bass_utils import run_bass_kernel_spmd
    res = run_bass_kernel_spmd(nc, in_maps, core_ids=list(range(NCORES)),
                               trace=_trace)
    kernel.last_result = res

    y = np.zeros((B, S, HID), np.float32)
    for c in range(NCORES):
        y[c // GRPS] += res.results[c]["y_t"].T
    return y


# revision 24
# speedup vs baseline: 1.0536x; 1.0536x over previous
"""Multi-head differential attention on 8 Trainium2 NeuronCores.

Sharding: data-parallel over batch (B=2) x tensor-parallel over heads
(16 heads -> 4 per core). Core c handles batch c//4 and heads
4*(c%4) .. 4*(c%4)+3. Each core computes its heads' attention output and a
partial output projection; the host sums the 4 partials per batch.

v5 over v2: softmax denominators accumulated elementwise on the vector
engine (one ones-matmul per (head, block) instead of one per key chunk,
-100K PE cycles); block-contiguous DRAM layouts with one ~2MB DMA per
block (each dma_start costs ~0.6us of issue time on its engine, so
per-chunk DMAs were issue-rate-bound at ~190GB/s); weights+consts on the
scalar DGE queue, x and y on the sync queue; warmup matmuls to ramp the
PE p-state during the initial DMA wait; kc-outer projection loops;
output projection spread one 512-col unit at a time between attention
iterations as PE filler; softmax-denominator + rms chains deferred two
iterations into the next head so the PE never parks on the DVE drain;
final block's projection via SBUF accumulators.
"""

import math
import os
import sys

sys.path.insert(0, "/opt/trn_rl_repo")

import numpy as np

B, S, HID, NH = 2, 2048, 2048, 16
HD = HID // NH          # 128
QKD = HD // 2           # 64
NCORES = 8
GRPS = NCORES // B      # head groups per batch
HPC = NH // GRPS        # heads per core = 4
LAYER_ID = 1
LAMBDA_INIT = 0.8 - 0.6 * math.exp(-0.3 * LAYER_ID)
EPS = 1e-6

NB = S // 512           # 4 seq blocks of 512
NKC = S // 128          # 16 key chunks of 128

_PROGRAM = None         # compiled bass program, reused across calls


def _build_program():
    import concourse.bass as bass
    import concourse.tile as tile
    from concourse import bacc, bass_isa, mybir

    f32 = mybir.dt.float32
    bf16 = mybir.dt.bfloat16
    Alu = mybir.AluOpType
    Act = mybir.ActivationFunctionType

    nc = bacc.Bacc(None, target_bir_lowering=False, debug=False)

    def din(name, shape, dt=bf16):
        return nc.dram_tensor(name, shape, dt, kind="ExternalInput").ap()

    io = {
        # block-contiguous: x row-block n*128+p, col kc*512+j holds
        # x_t[kc*128+p, n*512+j]; one 2MB dma per block, 16KB packet lines
        "xq_t": din("xq_t", [NB * 128, NKC * 512]),
        "xk_t": din("xk_t", [NB * 128, NKC * 512]),
        "xv_t": din("xv_t", [NB * 128, NKC * 512]),
        # chunk-contiguous: row p, col kc*512+j holds w_t[kc*128+p, j]
        "wq_t": din("wq_t", [128, NKC * 512]),
        "wk_t": din("wk_t", [128, NKC * 512]),
        "wv_t": din("wv_t", [128, NKC * 512]),
        "wo_t": din("wo_t", [512, HID]),
        "crep": din("crep", [128, S]),
        "srep": din("srep", [128, S]),
        "pmat": din("pmat", [128, 128]),
        "ones_a": din("ones_a", [128, 128]),
        "trimask": din("trimask", [128, 128]),
        "neglam": din("neglam", [128, 1], f32),
    }
    y_t = nc.dram_tensor("y_t", [HID, S], f32, kind="ExternalOutput").ap()

    from contextlib import ExitStack

    with tile.TileContext(nc) as tc, ExitStack() as ctx:
        persist = ctx.enter_context(tc.tile_pool(name="persist", bufs=1))
        constp = ctx.enter_context(tc.tile_pool(name="constp", bufs=1))

        # constants
        crep = constp.tile([128, S], bf16, name="crep_sb", tag="crep")
        srep = constp.tile([128, S], bf16, name="srep_sb", tag="srep")
        pmat = constp.tile([128, 128], bf16, name="pmat_sb", tag="pmat")
        ones_a = constp.tile([128, 128], bf16, name="ones_a_sb", tag="ones_a")
        trimask = constp.tile([128, 128], bf16, name="trimask_sb", tag="trimask")
        neglam = constp.tile([128, 1], f32, name="neglam_sb", tag="neglam")

        # persistent tensors: Q^T/K^T per (map g, head-pair hp): [128, S]
        #   tile t = 2*g + hp; partitions [64*a, 64*a+64) hold head 2*hp+a.
        QT = [persist.tile([128, S], bf16, name=f"qt{t}", tag=f"qt{t}")
              for t in range(4)]
        KT = [persist.tile([128, S], bf16, name=f"kt{t}", tag=f"kt{t}")
              for t in range(4)]
        # V natural layout per 128-seq chunk: [128 seq, 4 heads * 128 feat]
        VH = [persist.tile([128, 512], bf16, name=f"vh{s}", tag=f"vh{s}")
              for s in range(NKC)]
        # combined attention output (post RMS): [feat, seq] per head
        U = [persist.tile([128, S], bf16, name=f"u{h}", tag=f"u{h}")
             for h in range(HPC)]
        wo = [persist.tile([128, S], bf16, name=f"wo{h}", tag=f"wo{h}")
              for h in range(HPC)]

        # ---------------- phase W: warmup --------------------------------
        # the PE clock drops to a low p-state when idle and takes ~3us of
        # continuous execution to ramp back up; run junk matmuls on memset
        # tiles during the initial DMA wait so the first real matmuls run at
        # full speed (and the wait itself is hidden)
        with tc.tile_pool(name="warm", bufs=1) as warm, \
             tc.tile_pool(name="wps", bufs=1, space="PSUM") as wps:
            jnk = warm.tile([128, 512], bf16, name="jnk", tag="jnk")
            nc.vector.memset(jnk[:], 0.0)
            jp = wps.tile([128, 512], f32, name="jp", tag="jp")
            for i in range(12):
                nc.tensor.matmul(jp[:], jnk[:, 0:128], jnk[:],
                                 start=(i == 0), stop=(i == 11))

        # ---------------- phase P: q/k/v projections + fused rope ----------
        with tc.tile_pool(name="wp", bufs=1) as wp, \
             tc.tile_pool(name="xp", bufs=2) as xp, \
             tc.tile_pool(name="rsb", bufs=2) as rsb, \
             tc.tile_pool(name="rawp", bufs=6) as rawp, \
             tc.tile_pool(name="pp", bufs=1, space="PSUM") as pp, \
             tc.tile_pool(name="pxp", bufs=2, space="PSUM") as pxp:
            wnames = ("wq_t", "wk_t", "wv_t")

            # Each dma_start costs ~0.6us on the ISSUING engine (descriptor
            # prep), so DMA throughput is issue-rate-bound: use one big
            # descriptor per block (packets auto-spread across all 16 DMA
            # engines), quartered only where progressive arrival matters
            # (w0 and the first x block feed the very first matmuls).
            # Weights + consts on the scalar queue, x on the sync queue.
            def load_w(mi, tag, nsplit):
                w_ = wp.tile([128, NKC * 512], bf16, name=f"w{mi}", tag=tag)
                step = NKC * 512 // nsplit
                for qtr in range(nsplit):
                    sl = slice(qtr * step, (qtr + 1) * step)
                    nc.scalar.dma_start(out=w_[:, sl],
                                        in_=io[wnames[mi]][:, sl])
                return w_

            wb = {0: load_w(0, "wA", 4)}
            for t, key in ((pmat, "pmat"), (trimask, "trimask"),
                           (neglam, "neglam"), (ones_a, "ones_a"),
                           (crep, "crep"), (srep, "srep")):
                nc.scalar.dma_start(out=t[:], in_=io[key][:])
            for h in range(HPC):
                nc.scalar.dma_start(out=wo[h][:],
                                    in_=io["wo_t"][h * 128:(h + 1) * 128, :])
            wb[1] = load_w(1, "wB", 1)
            # reuses w0's buffer: the dma WAR-waits on mi=0's last matmul,
            # parking the scalar queue behind it (nothing else needs it then)
            wb[2] = load_w(2, "wA", 1)

            def load_x(mi, n, nsplit=1):
                xin = io[("xq_t", "xk_t", "xv_t")[mi]]
                xb = xp.tile([128, NKC * 512], bf16, name=f"x_{mi}_{n}",
                             tag="xb")
                step = NKC * 512 // nsplit
                for qtr in range(nsplit):
                    sl = slice(qtr * step, (qtr + 1) * step)
                    nc.sync.dma_start(out=xb[:, sl],
                                      in_=xin[n * 128:(n + 1) * 128, sl])
                return xb

            pending = []

            def flush_rope():
                T, t, n, raw = pending.pop(0)
                px = pxp.tile([128, 512], f32, name=f"px_{T[t].name}_{n}",
                              tag="px")
                nc.tensor.matmul(px[:], pmat[:], raw[:], start=True, stop=True)
                pxb = rsb.tile([128, 512], bf16, name=f"pxb_{T[t].name}_{n}",
                               tag="pxb")
                nc.scalar.copy(pxb[:], px[:])
                cs = slice(n * 512, (n + 1) * 512)
                tmp = rsb.tile([128, 512], bf16, name=f"tmp_{T[t].name}_{n}",
                               tag="tmp")
                nc.vector.tensor_mul(tmp[:], pxb[:], srep[:, cs])
                aa = rsb.tile([128, 512], bf16, name=f"aa_{T[t].name}_{n}",
                              tag="aa")
                nc.vector.tensor_mul(aa[:], raw[:], crep[:, cs])
                nc.vector.tensor_add(T[t][:, cs], aa[:], tmp[:])

            blocks = [(mi, n) for mi in range(3) for n in range(NB)]
            xts = {0: load_x(*blocks[0], nsplit=4),
                   1: load_x(*blocks[1], nsplit=4)}
            for idx, (mi, n) in enumerate(blocks):
                xt = xts.pop(idx)
                if idx + 2 < len(blocks):
                    xts[idx + 2] = load_x(*blocks[idx + 2])
                ps = [pp.tile([128, 512], f32, name=f"pp{t}_{mi}_{n}",
                              tag=f"pp{t}") for t in range(4)]
                w_ = wb[mi]
                # kc-outer: each x/w chunk is consumed in 4 back-to-back
                # matmuls right after it lands, so the PE never waits for a
                # whole 16-chunk block of DMA before starting
                for kc in range(NKC):
                    ck = slice(kc * 512, (kc + 1) * 512)
                    for t in range(4):
                        co = kc * 512 + t * 128
                        nc.tensor.matmul(
                            ps[t][:],
                            w_[:, co:co + 128]
                            if mi < 2 else xt[:, co:co + 128],
                            xt[:, ck] if mi < 2 else w_[:, ck],
                            start=(kc == 0), stop=(kc == 15))
                    if kc in (2, 6, 10, 14) and pending:
                        flush_rope()
                for t in range(4):
                    if mi == 2:
                        nc.scalar.copy(VH[n * 4 + t][:], ps[t][:])
                    else:
                        raw = rawp.tile([128, 512], bf16,
                                        name=f"raw_{mi}_{n}_{t}", tag="raw")
                        # on scalar: in the vector queue this cast sits
                        # behind rope multiplies, and the next block's
                        # psum-tag WAR stalls the PE on it
                        nc.scalar.copy(raw[:], ps[t][:])
                        pending.append((QT if mi == 0 else KT, t, n, raw))
            while pending:
                flush_rope()

        # ---------------- phase A: attention + rms + output proj ----------
        with tc.tile_pool(name="sp", bufs=2, space="PSUM") as sp, \
             tc.tile_pool(name="pvp", bufs=1, space="PSUM") as pvp, \
             tc.tile_pool(name="op", bufs=2, space="PSUM") as op, \
             tc.tile_pool(name="rsp", bufs=2) as rsp, \
             tc.tile_pool(name="ep", bufs=5) as ep, \
             tc.tile_pool(name="cb", bufs=2) as cb, \
             tc.tile_pool(name="ys", bufs=2) as ys:
            pending_rms = []
            pending_sm = []
            i32 = mybir.dt.int32

            def flush_rms():
                # rms tail for a completed (h, qb): partition-sum of U^2 on
                # the PE, then rstd = rsqrt(mean+eps) via the exponent
                # bit-trick + one Newton step on the vector engine.
                h_, qb_, sq_, dst_ = pending_rms.pop()
                ssq_t = op.tile([128, 512], f32, name=f"ssq_{h_}_{qb_}",
                                tag="o")
                nc.tensor.matmul(ssq_t[:], ones_a[:], sq_[:],
                                 start=True, stop=True)
                m_ = cb.tile([128, 512], f32, name=f"m_{h_}_{qb_}", tag="m")
                # on vector, not scalar: the scalar queue is exp-saturated in
                # phase A and any extra op there delays the exp chain, which
                # WAR-stalls the PE on the scores double-buffer
                nc.vector.tensor_scalar(m_[:], ssq_t[:], 1.0 / HD, EPS,
                                        op0=Alu.mult, op1=Alu.add)
                y0 = cb.tile([128, 512], f32, name=f"y0_{h_}_{qb_}", tag="y0")
                nc.vector.tensor_scalar(
                    y0[:].bitcast(i32), m_[:].bitcast(i32), 1, -1,
                    op0=Alu.logical_shift_right, op1=Alu.bitwise_xor)
                nc.vector.tensor_scalar(
                    y0[:].bitcast(i32), y0[:].bitcast(i32), 0x5f3759e0, None,
                    op0=Alu.add)
                uu = cb.tile([128, 512], f32, name=f"uu_{h_}_{qb_}", tag="uu")
                nc.vector.tensor_mul(uu[:], y0[:], y0[:])
                nc.vector.scalar_tensor_tensor(
                    uu[:], m_[:], -0.5, uu[:], op0=Alu.mult, op1=Alu.mult)
                nc.vector.tensor_scalar(uu[:], uu[:], 1.5, None, op0=Alu.add)
                rstdb = cb.tile([128, 512], bf16, name=f"rstdb_{h_}_{qb_}",
                                tag="rstdb")
                nc.vector.tensor_mul(rstdb[:], y0[:], uu[:])
                nc.vector.tensor_mul(dst_, dst_, rstdb[:])

            def oproj_unit(qb, pr, i):
                # one 128-row x 512-query slice of the output projection for
                # completed block qb; spread between attention iterations as
                # PE filler
                oc = 2 * pr + i
                psy = op.tile([128, 512], f32, name=f"py_{qb}_{oc}", tag="o")
                for h2 in range(HPC):
                    nc.tensor.matmul(
                        psy[:], wo[h2][:, oc * 128:(oc + 1) * 128],
                        U[h2][:, qb * 512:(qb + 1) * 512],
                        start=(h2 == 0), stop=(h2 == HPC - 1))
                yst = ys.tile([128, 512], f32, name=f"yst_{qb}_{oc}",
                              tag="yst")
                # psum->sbuf drain on vector, keeping scalar free for exp
                nc.vector.tensor_scalar(yst[:], psy[:], 0.0, None,
                                        op0=Alu.add)
                nc.sync.dma_start(
                    out=y_t[oc * 128:(oc + 1) * 128,
                            qb * 512:(qb + 1) * 512],
                    in_=yst[:])

            def flush_sm():
                # deferred softmax-denominator + combine chain for the
                # previous head, emitted two iterations into the next head's
                # loop so the PE never sits on the DVE drain at an h boundary
                h_, qb_, pv_, run_ = pending_sm.pop()
                rb = cb.tile([128, 1024], f32, name=f"rb_{h_}_{qb_}",
                             tag="rb")
                tt = cb.tile([128, 1024], f32, name=f"tt_{h_}_{qb_}",
                             tag="tt")
                for g in (0, 1):
                    sl = slice(g * 512, (g + 1) * 512)
                    # bf16 accumulator: each element only sums <=16 chunk
                    # values (the 128-key reduction happens in f32 psum), so
                    # rounding stays ~0.3% on the denominators
                    ps2 = op.tile([128, 512], f32, name=f"sm{g}_{h_}_{qb_}",
                                  tag="o")
                    nc.tensor.matmul(ps2[:], ones_a[:], run_[:, sl],
                                     start=True, stop=True)
                    nc.vector.reciprocal_approx_fast(rb[:, sl], ps2[:])
                    nc.vector.tensor_mul(tt[:, sl], pv_[:, sl], rb[:, sl])
                dst = U[h_][:, qb_ * 512:(qb_ + 1) * 512]
                nc.vector.scalar_tensor_tensor(
                    dst, tt[:, 512:1024], neglam[:], tt[:, 0:512],
                    op0=Alu.mult, op1=Alu.add)
                sq = cb.tile([128, 512], bf16, name=f"sq_{h_}_{qb_}",
                             tag="sq")
                nc.vector.tensor_mul(sq[:], dst, dst)
                pending_rms.append((h_, qb_, sq, dst))

            oq = []           # pending oproj units of the previous qb
            for qb in range(NB):
                nkc = 4 * qb + 4
                iters = HPC * nkc
                due = []
                if oq:
                    due = [7 + round(u * (iters - 9) / len(oq))
                           for u in range(len(oq))]
                it = 0
                for h in range(HPC):
                    hp, a = h // 2, h % 2
                    poff = 64 * a
                    pv = pvp.tile([128, 1024], f32, name=f"pv_{h}_{qb}",
                                  tag="pv")
                    run = rsp.tile([128, 1024], bf16, name=f"run_{h}_{qb}",
                                   tag="run")
                    pvq = []

                    def emit_pv(item):
                        E_, qoff_, first_, last_, kc_ = item
                        for g in (0, 1):
                            sl = slice(g * 512 + qoff_, (g + 1) * 512)
                            nc.tensor.matmul(
                                pv[:, sl],
                                VH[kc_][:, h * 128:(h + 1) * 128],
                                E_[:, sl], start=first_, stop=last_)

                    for kc in range(nkc):
                        j = kc - 4 * qb
                        qoff = max(j, 0) * 128
                        if kc == 1 and pending_sm:
                            flush_sm()
                        if kc == 3 and pending_rms:
                            flush_rms()
                        while oq and due and it >= due[0]:
                            due.pop(0)
                            oproj_unit(*oq.pop(0))
                        ps = sp.tile([128, 1024], f32, name=f"s_{h}_{qb}_{kc}",
                                     tag="s")
                        for g in (0, 1):
                            tq = 2 * g + hp
                            nc.tensor.matmul(
                                ps[:, g * 512 + qoff:(g + 1) * 512],
                                KT[tq][poff:poff + 64,
                                       kc * 128:(kc + 1) * 128],
                                QT[tq][poff:poff + 64,
                                       qb * 512 + qoff:(qb + 1) * 512],
                                start=True, stop=True)
                        E = ep.tile([128, 1024], bf16, name=f"e_{h}_{qb}_{kc}",
                                    tag="e")
                        if qoff == 0:
                            nc.scalar.activation(E[:], ps[:], Act.Exp,
                                                 scale=0.125)
                        else:
                            for g in (0, 1):
                                nc.scalar.activation(
                                    E[:, g * 512 + qoff:(g + 1) * 512],
                                    ps[:, g * 512 + qoff:(g + 1) * 512],
                                    Act.Exp, scale=0.125)
                        if j >= 0:
                            for g in (0, 1):
                                sl = E[:, g * 512 + qoff:g * 512 + qoff + 128]
                                # causal mask on the idle GpSimd engine
                                nc.gpsimd.tensor_mul(sl, sl, trimask[:])
                        # softmax denominator: elementwise chunk accumulation
                        # on the vector engine; partition-sum once per (h,qb)
                        if kc == 0:
                            nc.vector.tensor_scalar(run[:], E[:], 0.0, None,
                                                    op0=Alu.add)
                        elif qoff == 0:
                            nc.vector.tensor_add(run[:], run[:], E[:])
                        else:
                            for g in (0, 1):
                                sl = slice(g * 512 + qoff, (g + 1) * 512)
                                nc.vector.tensor_add(run[:, sl], run[:, sl],
                                                     E[:, sl])
                        pvq.append((E, qoff, kc == 0, kc == nkc - 1, kc))
                        if len(pvq) > 3:
                            emit_pv(pvq.pop(0))
                        it += 1
                    while pvq:
                        emit_pv(pvq.pop(0))
                    # combine U = pv1/sm1 - lam*pv2/sm2 is deferred into the
                    # next head's loop (flush_sm)
                    pending_sm.append((h, qb, pv, run))
                while oq:
                    oproj_unit(*oq.pop(0))
                if qb < NB - 1:
                    oq = [(qb, pr, i) for pr in range(8) for i in (0, 1)]
            # final block's output projection, restructured so the PE
            # never parks behind the h3 sm+rms DVE chains: 16 h0-h2 partial
            # chains stream into SBUF accumulators (scalar drains psum)
            # while the DVE chains run, then 16 single-matmul h3
            # completions + vector adds + DMAs close the kernel.
            qb = NB - 1
            with tc.tile_pool(name="yap", bufs=1) as yap:
                ya = []
                for oc in range(16):
                    psy = op.tile([128, 512], f32, name=f"pyp_{oc}", tag="o")
                    for h2 in range(HPC - 1):
                        nc.tensor.matmul(
                            psy[:], wo[h2][:, oc * 128:(oc + 1) * 128],
                            U[h2][:, qb * 512:(qb + 1) * 512],
                            start=(h2 == 0), stop=(h2 == HPC - 2))
                    t_ = yap.tile([128, 512], f32, name=f"ya{oc}",
                                  tag=f"ya{oc}")
                    nc.scalar.copy(t_[:], psy[:])
                    ya.append(t_)
                    if oc == 1 and pending_sm:
                        flush_sm()
                    if oc == 3 and pending_rms:
                        flush_rms()
                for oc in range(16):
                    psy = op.tile([128, 512], f32, name=f"pyh3_{oc}", tag="o")
                    nc.tensor.matmul(
                        psy[:], wo[HPC - 1][:, oc * 128:(oc + 1) * 128],
                        U[HPC - 1][:, qb * 512:(qb + 1) * 512],
                        start=True, stop=True)
                    nc.vector.tensor_add(ya[oc][:], ya[oc][:], psy[:])
                    eng = nc.sync if oc % 2 == 0 else nc.scalar
                    eng.dma_start(
                        out=y_t[oc * 128:(oc + 1) * 128,
                                qb * 512:(qb + 1) * 512],
                        in_=ya[oc][:])

    nc.compile()
    return nc


def _host_prep(q, k, v, Wq, Wk, Wv, Wo, lambda_q1, lambda_k1, lambda_q2,
               lambda_k2, gnorm_w, cos_emb, sin_emb):
    import ml_dtypes

    f32 = np.float32
    bf16 = ml_dtypes.bfloat16
    q = np.asarray(q, f32); k = np.asarray(k, f32); v = np.asarray(v, f32)
    Wq = np.asarray(Wq, f32); Wk = np.asarray(Wk, f32)
    Wv = np.asarray(Wv, f32); Wo = np.asarray(Wo, f32)
    gnorm_w = np.asarray(gnorm_w, f32)
    cos_emb = np.asarray(cos_emb, f32); sin_emb = np.asarray(sin_emb, f32)

    lam1 = np.exp(np.sum(np.asarray(lambda_q1, f32) * np.asarray(lambda_k1, f32),
                         dtype=f32))
    lam2 = np.exp(np.sum(np.asarray(lambda_q2, f32) * np.asarray(lambda_k2, f32),
                         dtype=f32))
    lam = np.float32(lam1 - lam2 + LAMBDA_INIT)

    # per-batch transposed activations (bf16), block-contiguous:
    # xb[n*128+p, kc*512+j] = x.T[kc*128+p, n*512+j]
    def blockify_x(xT):
        r = xT.reshape(NKC, 128, NB, 512).transpose(2, 1, 0, 3)
        return np.ascontiguousarray(r.reshape(NB * 128, NKC * 512))

    xt = {}
    for b in range(B):
        xt[("q", b)] = blockify_x(q[b].T.astype(bf16))
        xt[("k", b)] = blockify_x(k[b].T.astype(bf16))
        xt[("v", b)] = blockify_x(v[b].T.astype(bf16))

    # shared constant tensors
    base_c = cos_emb[:S, :QKD]          # [S, 64]
    base_s = sin_emb[:S, :QKD]
    crep = np.ascontiguousarray(np.tile(base_c.T, (2, 1))).astype(bf16)
    srep = np.ascontiguousarray(np.tile(base_s.T, (2, 1))).astype(bf16)
    pmat = np.zeros((128, 128), f32)
    for blk in range(2):
        o = blk * 64
        for i in range(QKD // 2):
            pmat[o + 2 * i, o + 2 * i + 1] = 1.0     # lhsT[2i, 2i+1]
            pmat[o + 2 * i + 1, o + 2 * i] = -1.0    # lhsT[2i+1, 2i]
    pmat = pmat.astype(bf16)
    ones_a = np.ones((128, 128), f32).astype(bf16)
    # trimask[p, n] = 1 if key-in-chunk p <= query-in-block n (valid)
    trimask = np.triu(np.ones((128, 128), f32), 0).astype(bf16)
    neglam = np.full((128, 1), -lam, f32)

    per_core = []
    for c in range(NCORES):
        b, grp = c // GRPS, c % GRPS
        heads = [HPC * grp + j for j in range(HPC)]
        # wq/wk columns: tile t = 2*g + hp; within tile: head 2*hp+a at
        # cols [64*a, 64*a+64), original feature order (interleaved pairs)
        cols = []
        for t in range(4):
            g, hp = t // 2, t % 2
            for a2 in range(2):
                hg = heads[2 * hp + a2]
                cols.extend(hg * HD + g * QKD + d for d in range(QKD))
        cols = np.asarray(cols)
        vrows = np.asarray([h * HD + d for h in heads for d in range(HD)])
        # chunk-contiguous weights: wb[p, kc*512+j] = w_t[kc*128+p, j]
        def blockify_w(wT):
            return np.ascontiguousarray(
                wT.reshape(NKC, 128, 512).transpose(1, 0, 2).reshape(
                    128, NKC * 512))

        wq_t = blockify_w(Wq[cols, :].T.astype(bf16))
        wk_t = blockify_w(Wk[cols, :].T.astype(bf16))
        wv_t = blockify_w(Wv[vrows, :].T.astype(bf16))
        gtile = np.tile(gnorm_w, HPC)                       # [512]
        wo_t = np.ascontiguousarray(
            ((1.0 - LAMBDA_INIT) * Wo[:, vrows] * gtile[None, :]).T).astype(bf16)
        per_core.append({
            "xq_t": xt[("q", b)], "xk_t": xt[("k", b)], "xv_t": xt[("v", b)],
            "wq_t": wq_t, "wk_t": wk_t, "wv_t": wv_t, "wo_t": wo_t,
            "crep": crep, "srep": srep, "pmat": pmat,
            "ones_a": ones_a, "trimask": trimask, "neglam": neglam,
        })
    return per_core


def _install_ntff_hook():
    """antenv.axon_hooks is absent in this image; synthesize it so
    run_bass_kernel_spmd(trace=True) can capture NTFF profiles."""
    import sys as _sys
    import types

    if "antenv.axon_hooks" in _sys.modules:
        return
    import antenv
    mod = types.ModuleType("antenv.axon_hooks")
    state = {"hook": None}
    mod.set_axon_ntff_profile_hook = lambda h: state.__setitem__("hook", h)
    mod.get_axon_ntff_profile_hook = lambda: state["hook"]
    _sys.modules["antenv.axon_hooks"] = mod
    antenv.axon_hooks = mod
    try:
        from trn_agent_boot.trn_boot import _ntff_profile_via_ctypes
        state["hook"] = _ntff_profile_via_ctypes("/opt/axon/libaxon_pjrt.so")
    except Exception as e:  # degrade: trace skipped, run still works
        print("ntff hook install failed:", e)


def kernel(q, k, v, Wq, Wk, Wv, Wo, lambda_q1, lambda_k1, lambda_q2,
           lambda_k2, gnorm_w, cos_emb, sin_emb, mask, _trace=False):
    if _trace:
        _install_ntff_hook()
    global _PROGRAM
    if _PROGRAM is None:
        _PROGRAM = _build_program()
    nc = _PROGRAM

    in_maps = _host_prep(q, k, v, Wq, Wk, Wv, Wo, lambda_q1, lambda_k1,
                         lambda_q2, lambda_k2, gnorm_w, cos_emb, sin_emb)

    from concourse.bass_utils import run_bass_kernel_spmd
    res = run_bass_kernel_spmd(nc, in_maps, core_ids=list(range(NCORES)),
                               trace=_trace)
    kernel.last_result = res

    y = np.zeros((B, S, HID), np.float32)
    for c in range(NCORES):
        y[c // GRPS] += res.results[c]["y_t"].T
    return y
